# revision 13
# baseline (speedup 1.0000x reference)
"""Based-style linear attention (Taylor feature map) on 8 Trainium2 cores.

Math: reference computes, per head h (FDIM=16, HEAD_DIM=64):
    q,k = HS@Wq, HS@Wk    (per-head 16 dims), v = HS@Wv (per-head 64 dims)
    phi(x) = [1, x/2, outer(x,x)/(sqrt(2)*4)]      (273 dims)
    y_t = sum_{s<=t} (phi(q_t).phi(k_s)) v_s / sum_{s<=t} phi(q_t).phi(k_s)
    out = concat_h(y) @ Wo
Key identity: phi(q).phi(k) = Square(S/sqrt(32) + 1/sqrt(2)) + 1/2, S = q.k.

Sharding: head-parallel, no collectives. 16 virtual heads (12 real + 4
zero dummies), 2 per core. Host sums the 8 partial outputs.

v3 (vs v1 baseline 66.6us, v2 78.5us):
 - Warmup: K=128 dummy matmuls into the proj psum banks during the input
   DMA wait keep the HAM clock gate at 8/8 (2.4 GHz); v1/v2 ran half the
   kernel at 1.2 GHz. (K=1 dummies in v2 didn't register as PE activity.)
 - v-proj: wv-stationary (12 big N=512 MMs + 8 PE transposes) instead of
   48 N=130 MMs with 48 128-col LDWEIGHTS (LDW serialized ~5us in v1).
 - intra-chunk +1/2 causal term folded into the diagonal DVE mask ops
   (kills 16 N=128 htri matmuls); inter-chunk term via 4 N=512 sel MMs.
 - recip/broadcast/div batched per half-L (N=512 everywhere).
 - o-proj transposed: out^T[e,t] with wo-stationary, 24 uniform N=512
   MMs; output leaves as out^T via 2 DMA issues; host transposes.
 - big-N MM stream throughout keeps the PE duty cycle high so the HAM
   clock gate stays released (v1/v2's small-MM phases re-throttled it).
"""

import math

import numpy as np
import ml_dtypes

import concourse.bass as bass
import concourse.mybir as mybir
import concourse.tile as tile
from concourse import bacc
from concourse.bass_utils import run_bass_kernel_spmd

L = 1024
D = 768
H = 12
FD = 16
HD = 64
NCORE = 8
NCH = 8  # L chunks of 128
KB = 6  # contraction blocks of 128 over D
F32 = mybir.dt.float32
BF16 = mybir.dt.bfloat16

DT_PROJ = BF16
DT_ATT = BF16
DT_OUT = BF16

A_SCALE = 1.0 / math.sqrt(32.0)
A_BIAS = 1.0 / math.sqrt(2.0)

N_WARM = 8
N_WARM_COLS = 400

_compiled_nc = None
_last_in_maps = None


def _np_dt(dt):
    return ml_dtypes.bfloat16 if dt == BF16 else np.float32


def _bank_splits(lo, hi, bank=512):
    """Split [lo, hi) at multiples of `bank` (PSUM bank boundaries)."""
    out = []
    a = lo
    while a < hi:
        b = min(hi, (a // bank + 1) * bank)
        out.append((a, b))
        a = b
    return out


def _build_nc():
    nc = bacc.Bacc("TRN2", target_bir_lowering=False, debug=False, num_devices=NCORE)

    hsT = nc.dram_tensor("hsT", [D, L], DT_PROJ, kind="ExternalInput")
    wqv = nc.dram_tensor("wqv", [D, 256], DT_PROJ, kind="ExternalInput")
    wo = nc.dram_tensor("wo", [128, D], DT_OUT, kind="ExternalInput")
    # consts packed: tri 0:128 | ident 128:256 | ones8 256:320 | sel 320:1344
    c_all = nc.dram_tensor("c_all", [128, 1344], DT_ATT, kind="ExternalInput")
    outT = nc.dram_tensor("outT", [D, L], DT_OUT, kind="ExternalOutput")

    with tile.TileContext(nc) as tc:
        with (
            tc.tile_pool(name="cst", bufs=1) as cst,
            tc.tile_pool(name="sqp", bufs=4) as sqp,
            tc.tile_pool(name="wrk", bufs=2) as wrk,
        ):
            # ---- warmup fodder (no data deps; scheduled first) ----
            warm_st = cst.tile([128, 64], DT_ATT, tag="warm_st")
            warm_mv = cst.tile([128, N_WARM_COLS], DT_ATT, tag="warm_mv")
            nc.vector.memset(warm_st, 0.0)
            nc.vector.memset(warm_mv, 0.0)

            # ---- input DMAs: hs on sync queue, weights/consts on scalar ----
            wqv_re = wqv.ap().rearrange("(po pi) f -> pi po f", pi=128)
            hs_re = hsT.ap().rearrange("(po pi) f -> pi po f", pi=128)
            hs_sb = cst.tile([128, KB, L], DT_PROJ, tag="hs")
            nc.sync.dma_start(out=hs_sb[:, 0:1, :], in_=hs_re[:, 0:1, :])
            nc.sync.dma_start(out=hs_sb[:, 1:3, :], in_=hs_re[:, 1:3, :])
            nc.sync.dma_start(out=hs_sb[:, 3:6, :], in_=hs_re[:, 3:6, :])
            wqv_sb = cst.tile([128, KB, 256], DT_PROJ, tag="wqv")
            nc.scalar.dma_start(out=wqv_sb, in_=wqv_re)
            wk_sb = wqv_sb[:, :, 0:64]
            wq_sb = wqv_sb[:, :, 64:128]
            wv_sb = wqv_sb[:, :, 128:256]
            call_sb = cst.tile([128, 1344], DT_ATT, tag="call")
            nc.scalar.dma_start(out=call_sb, in_=c_all.ap())
            tri_sb = call_sb[:, 0:128]
            ident_sb = call_sb[:, 128:256]
            ones8_sb = call_sb[:, 256:320]
            sel_sb = call_sb[0:8, 320:1344]
            # wo as two base-0 tiles (o-proj lhsT per head)
            wo_sb = []
            for h in range(2):
                t = cst.tile([64, D], DT_OUT, tag=f"wo{h}", name=f"wo{h}")
                nc.scalar.dma_start(out=t, in_=wo.ap()[64 * h : 64 * h + 64, :])
                wo_sb.append(t)
            bias_sb = cst.tile([128, 1], F32, tag="bias")
            nc.vector.memset(bias_sb, A_BIAS)
            # row of ones at partition 64, for the den-reciprocal broadcast
            ones64_sb = cst.tile([65, 64], F32, tag="ones64")
            nc.vector.memset(ones64_sb, 0.0)
            nc.vector.memset(ones64_sb[64:65, :], 1.0)

            kq_sb = cst.tile([64, 2048], DT_ATT, tag="kq")
            vT_sb = cst.tile([128, L], DT_ATT, tag="vT")
            vx_sb = cst.tile([128, NCH, 130], DT_ATT, tag="vx")
            colsum_sb = cst.tile([8, 130], DT_ATT, tag="colsum")
            yT_sb = [
                cst.tile([64, L], DT_OUT, tag=f"yT{h}", name=f"yT{h}") for h in range(2)
            ]
            outT_sb = cst.tile([128, KB, L], DT_OUT, tag="outT")

            # ================= warmup + projections =================
            with tc.tile_pool(name="ps1", bufs=1, space="PSUM") as ps1:
                pqk = {}
                for half in range(2):
                    for coff in (0, 1024):
                        pqk[(half, coff)] = ps1.tile(
                            [64, 512],
                            F32,
                            tag=f"pqk{coff}_{half}",
                            name=f"pqk{coff}_{half}",
                        )
                pvT = [
                    ps1.tile([128, 512], F32, tag=f"pvT{i}", name=f"pvT{i}")
                    for i in range(2)
                ]
                ptp = ps1.tile([128, 128], DT_ATT, tag="ptp", name="ptp")
                pcs = ps1.tile([8, 130], F32, tag="pcs", name="pcs")

                # dummy matmuls into the qk psum banks: real K=128 array
                # activity that trips the HAM clock gate to 8/8 while the
                # input DMAs land. Overwritten by the first real qk MM.
                slots = list(pqk.values())
                for i in range(N_WARM):
                    nc.tensor.matmul(
                        slots[i % 4][:, 0:N_WARM_COLS],
                        warm_st,
                        warm_mv,
                        start=True,
                        stop=True,
                    )

                # q/k -> kq_sb [64, 2048]; partitions 0-15 head0, 32-47
                # head1 (rest zero); cols 0-1023 = k^T, 1024-2047 = q^T
                for kb in range(KB):
                    for half in range(2):
                        for w_sb, coff in ((wk_sb, 0), (wq_sb, 1024)):
                            nc.tensor.matmul(
                                pqk[(half, coff)],
                                w_sb[:, kb, :],
                                hs_sb[:, kb, half * 512 : (half + 1) * 512],
                                start=(kb == 0),
                                stop=(kb == KB - 1),
                            )
                    # v^T accumulation: wv-stationary, [c=128, t] layout
                    for half in range(2):
                        nc.tensor.matmul(
                            pvT[half],
                            wv_sb[:, kb, :],
                            hs_sb[:, kb, half * 512 : (half + 1) * 512],
                            start=(kb == 0),
                            stop=(kb == KB - 1),
                        )
                for half in range(2):
                    for coff in (0, 1024):
                        nc.vector.tensor_copy(
                            kq_sb[:, coff + half * 512 : coff + (half + 1) * 512],
                            pqk[(half, coff)],
                        )
                    nc.vector.tensor_copy(
                        vT_sb[:, half * 512 : (half + 1) * 512], pvT[half]
                    )

                # transpose v^T -> vx [kv, c] per 128-chunk; ones cols ride
                # at 64 / 129 for the denominator
                nc.vector.memset(vx_sb[:, :, 64], 1.0)
                nc.vector.memset(vx_sb[:, :, 129], 1.0)
                for ch in range(NCH):
                    nc.tensor.transpose(
                        ptp, vT_sb[:, ch * 128 : (ch + 1) * 128], ident_sb
                    )
                    nc.any.tensor_copy(vx_sb[:, ch, 0:64], ptp[:, 0:64])
                    nc.any.tensor_copy(vx_sb[:, ch, 65:129], ptp[:, 64:128])

                # per-chunk column sums of vx (inter-chunk +1/2 term)
                for ch in range(NCH):
                    nc.tensor.matmul(
                        pcs,
                        ones8_sb[:, ch * 8 : (ch + 1) * 8],
                        vx_sb[:, ch, :],
                        start=(ch == 0),
                        stop=(ch == NCH - 1),
                    )
                nc.vector.tensor_copy(colsum_sb, pcs)

            # ================= attention, half-L streamed =================
            with (
                tc.tile_pool(name="psnum", bufs=1, space="PSUM") as psnum,
                tc.tile_pool(name="psa", bufs=2, space="PSUM") as psa,
                tc.tile_pool(name="pso", bufs=2, space="PSUM") as pso,
            ):
                nums = [
                    psnum.tile([65, L], F32, tag=f"pN{h}", name=f"num{h}")
                    for h in range(2)
                ]

                def chunk_scores(j):
                    """Scores/num contributions of kv-chunk j to all t >=
                    j*128, including the intra-chunk +1/2 causal term (via
                    the diagonal-block DVE ops)."""
                    tlo = j * 128
                    for h in range(2):
                        sq = sqp.tile([128, 1024], DT_ATT, tag="sq", name=f"sq{j}_{h}")
                        for a, b in _bank_splits(tlo, L):
                            w = b - a
                            pa = psa.tile(
                                [128, 512], F32, tag="pA", name=f"pa{j}_{h}_{a}"
                            )[:, :w]
                            nc.tensor.matmul(
                                pa,
                                kq_sb[32 * h : 32 * h + 32, tlo : tlo + 128],
                                kq_sb[32 * h : 32 * h + 32, 1024 + a : 1024 + b],
                                start=True,
                                stop=True,
                            )
                            sqs = sq[:, a - tlo : b - tlo]
                            nc.scalar.activation(
                                out=sqs,
                                in_=pa,
                                func=mybir.ActivationFunctionType.Square,
                                scale=A_SCALE,
                                bias=bias_sb,
                            )
                            if a == tlo:
                                # diagonal block: causal mask + the +1/2
                                # intra-chunk term, (sq + 1/2) * tri
                                nc.vector.scalar_tensor_tensor(
                                    out=sqs[:, 0:128],
                                    in0=sqs[:, 0:128],
                                    scalar=0.5,
                                    in1=tri_sb,
                                    op0=mybir.AluOpType.add,
                                    op1=mybir.AluOpType.mult,
                                )
                            # num^T += V_j^T-stationary @ sq
                            nc.tensor.matmul(
                                nums[h][:, a:b],
                                vx_sb[:, j, 65 * h : 65 * h + 65],
                                sqs,
                                start=(j == 0),
                                stop=False,
                            )

                def half_finalize_num(half):
                    """Close the num accumulation for t-half `half`: add the
                    inter-chunk +1/2 term, then divide by den (row 64)."""
                    lo = half * 512
                    for h in range(2):
                        nc.tensor.matmul(
                            nums[h][:, lo : lo + 512],
                            colsum_sb[:, 65 * h : 65 * h + 65],
                            sel_sb[:, lo : lo + 512],
                            start=False,
                            stop=True,
                        )
                        rc = wrk.tile([65, 512], F32, tag="rc")
                        nc.vector.reciprocal_approx_fast(
                            out=rc, in_=nums[h][:, lo : lo + 512]
                        )
                        prb = pso.tile([128, 512], F32, tag="po", name=f"prb{half}_{h}")
                        nc.tensor.matmul(
                            prb[0:64, :],
                            ones64_sb[64:65, :],
                            rc[64:65, :],
                            start=True,
                            stop=True,
                        )
                        rb = wrk.tile([64, 512], F32, tag="rb")
                        nc.any.tensor_copy(rb, prb[0:64, :])
                        nc.vector.tensor_mul(
                            yT_sb[h][:, lo : lo + 512],
                            nums[h][0:64, lo : lo + 512],
                            rb,
                        )

                def half_oproj(half):
                    """out^T[e, t-half] = sum_h Wo_h^T yT_h; stream to HBM."""
                    lo = half * 512
                    for e in range(KB):
                        po = pso.tile([128, 512], F32, tag="po", name=f"po{half}_{e}")
                        for h in range(2):
                            nc.tensor.matmul(
                                po,
                                wo_sb[h][:, e * 128 : (e + 1) * 128],
                                yT_sb[h][:, lo : lo + 512],
                                start=(h == 0),
                                stop=(h == 1),
                            )
                        nc.any.tensor_copy(outT_sb[:, e, lo : lo + 512], po)
                    outT_re = outT.ap().rearrange("(po pi) t -> pi po t", pi=128)
                    nc.sync.dma_start(
                        out=outT_re[:, :, lo : lo + 512],
                        in_=outT_sb[:, :, lo : lo + 512],
                    )

                # software pipeline: finalize each t-half as soon as its last
                # kv-chunk lands, o-proj overlapped with later score chunks
                for j in range(NCH):
                    chunk_scores(j)
                    if j == 4:
                        half_finalize_num(0)
                    if j == 5:
                        half_oproj(0)
                half_finalize_num(1)
                half_oproj(1)

    nc.finalize()
    return nc


def _host_consts():
    s = np.arange(128)[:, None]
    t = np.arange(128)[None, :]
    tri = (s <= t).astype(np.float32)
    sel = np.zeros((8, 1024), dtype=np.float32)
    for i in range(8):
        sel[:i, i * 128 : (i + 1) * 128] = 0.5
    ident = np.eye(128, dtype=np.float32)
    ones8 = np.zeros((128, 64), dtype=np.float32)
    for ch in range(8):
        ones8[:, ch * 8 + ch] = 1.0
    return tri, sel, ident, ones8


def kernel(hidden_states, Wq, Wk, Wv, Wo):
    global _compiled_nc, _last_in_maps
    hs = np.asarray(hidden_states, dtype=np.float32)[0]  # [L, D]
    Wq = np.asarray(Wq, dtype=np.float32)
    Wk = np.asarray(Wk, dtype=np.float32)
    Wv = np.asarray(Wv, dtype=np.float32)
    Wo = np.asarray(Wo, dtype=np.float32)

    if _compiled_nc is None:
        _compiled_nc = _build_nc()
    nc = _compiled_nc

    proj_dt = _np_dt(DT_PROJ)
    att_dt = _np_dt(DT_ATT)
    out_dt = _np_dt(DT_OUT)

    hsT = np.ascontiguousarray(hs.T).astype(proj_dt)  # [D, L]
    tri, sel, ident, ones8 = _host_consts()
    c_all = np.zeros((128, 1344), dtype=np.float32)
    c_all[:, 0:128] = tri
    c_all[:, 128:256] = ident
    c_all[:, 256:320] = ones8
    c_all[0:8, 320:1344] = sel
    c_all = c_all.astype(att_dt)

    in_maps = []
    for c in range(NCORE):
        heads = [2 * c, 2 * c + 1]
        wk_c = np.zeros((D, 64), dtype=np.float32)
        wq_c = np.zeros((D, 64), dtype=np.float32)
        wv_c = np.zeros((D, 128), dtype=np.float32)
        wo_c = np.zeros((128, D), dtype=np.float32)
        for hi, h in enumerate(heads):
            if h >= H:
                continue
            wk_c[:, 32 * hi : 32 * hi + FD] = Wk[:, h * FD : (h + 1) * FD]
            wq_c[:, 32 * hi : 32 * hi + FD] = Wq[:, h * FD : (h + 1) * FD]
            wv_c[:, 64 * hi : 64 * hi + HD] = Wv[:, h * HD : (h + 1) * HD]
            wo_c[64 * hi : 64 * hi + HD, :] = Wo[h * HD : (h + 1) * HD, :]
        wqv_c = np.concatenate([wk_c, wq_c, wv_c], axis=1)
        in_maps.append(
            {
                "hsT": hsT,
                "wqv": wqv_c.astype(proj_dt),
                "wo": wo_c.astype(out_dt),
                "c_all": c_all,
            }
        )

    _last_in_maps = in_maps
    res = run_bass_kernel_spmd(nc, in_maps, list(range(NCORE)))
    acc = np.zeros((L, D), dtype=np.float32)
    for c in range(NCORE):
        acc += np.asarray(res.results[c]["outT"], dtype=np.float32).T
    return acc.reshape(1, L, D)


# revision 22
# speedup vs baseline: 1.0103x; 1.0103x over previous
"""Based-style linear attention (Taylor feature map) on 8 Trainium2 cores.

Math: reference computes, per head h (FDIM=16, HEAD_DIM=64):
    q,k = HS@Wq, HS@Wk    (per-head 16 dims), v = HS@Wv (per-head 64 dims)
    phi(x) = [1, x/2, outer(x,x)/(sqrt(2)*4)]      (273 dims)
    y_t = sum_{s<=t} (phi(q_t).phi(k_s)) v_s / sum_{s<=t} phi(q_t).phi(k_s)
    out = concat_h(y) @ Wo
Key identity: phi(q).phi(k) = Square(S/sqrt(32) + 1/sqrt(2)) + 1/2, S = q.k.

Sharding: head-parallel, no collectives. 16 virtual heads (12 real + 4
zero dummies), 2 per core. Host sums the 8 partial outputs.

v3 (vs v1 baseline 66.6us, v2 78.5us):
 - Warmup: K=128 dummy matmuls into the proj psum banks during the input
   DMA wait keep the HAM clock gate at 8/8 (2.4 GHz); v1/v2 ran half the
   kernel at 1.2 GHz. (K=1 dummies in v2 didn't register as PE activity.)
 - v-proj: wv-stationary (12 big N=512 MMs + 8 PE transposes) instead of
   48 N=130 MMs with 48 128-col LDWEIGHTS (LDW serialized ~5us in v1).
 - intra-chunk +1/2 causal term folded into the diagonal DVE mask ops
   (kills 16 N=128 htri matmuls); inter-chunk term via 4 N=512 sel MMs.
 - recip/broadcast/div batched per half-L (N=512 everywhere).
 - o-proj transposed: out^T[e,t] with wo-stationary, 24 uniform N=512
   MMs; output leaves as out^T via 2 DMA issues; host transposes.
 - big-N MM stream throughout keeps the PE duty cycle high so the HAM
   clock gate stays released (v1/v2's small-MM phases re-throttled it).
"""

import math

import numpy as np
import ml_dtypes

import concourse.bass as bass
import concourse.mybir as mybir
import concourse.tile as tile
from concourse import bacc
from concourse.bass_utils import run_bass_kernel_spmd

L = 1024
D = 768
H = 12
FD = 16
HD = 64
NCORE = 8
NCH = 8  # L chunks of 128
KB = 6  # contraction blocks of 128 over D
F32 = mybir.dt.float32
BF16 = mybir.dt.bfloat16

DT_PROJ = BF16
DT_ATT = BF16
DT_OUT = BF16

A_SCALE = 1.0 / math.sqrt(32.0)
A_BIAS = 1.0 / math.sqrt(2.0)

N_WARM = 7
N_WARM_COLS = 400

_compiled_nc = None
_last_in_maps = None


def _np_dt(dt):
    return ml_dtypes.bfloat16 if dt == BF16 else np.float32


def _bank_splits(lo, hi, bank=512):
    """Split [lo, hi) at multiples of `bank` (PSUM bank boundaries)."""
    out = []
    a = lo
    while a < hi:
        b = min(hi, (a // bank + 1) * bank)
        out.append((a, b))
        a = b
    return out


def _build_nc():
    nc = bacc.Bacc("TRN2", target_bir_lowering=False, debug=False, num_devices=NCORE)

    hsT = nc.dram_tensor("hsT", [D, L], DT_PROJ, kind="ExternalInput")
    wqv = nc.dram_tensor("wqv", [D, 256], DT_PROJ, kind="ExternalInput")
    wo = nc.dram_tensor("wo", [128, D], DT_OUT, kind="ExternalInput")
    # consts packed: tri 0:128 | ident 128:256 | ones8 256:320 | sel 320:1344
    c_all = nc.dram_tensor("c_all", [128, 1344], DT_ATT, kind="ExternalInput")
    outT = nc.dram_tensor("outT", [D, L], DT_OUT, kind="ExternalOutput")

    with tile.TileContext(nc) as tc:
        with (
            tc.tile_pool(name="cst", bufs=1) as cst,
            tc.tile_pool(name="sqp", bufs=4) as sqp,
            tc.tile_pool(name="wrk", bufs=2) as wrk,
        ):
            # ---- warmup fodder (no data deps; scheduled first) ----
            warm_st = cst.tile([128, 64], DT_ATT, tag="warm_st")
            warm_mv = cst.tile([128, N_WARM_COLS], DT_ATT, tag="warm_mv")
            nc.vector.memset(warm_st, 0.0)
            nc.vector.memset(warm_mv, 0.0)

            # ---- input DMAs: hs on sync queue, weights/consts on scalar ----
            wqv_re = wqv.ap().rearrange("(po pi) f -> pi po f", pi=128)
            hs_re = hsT.ap().rearrange("(po pi) f -> pi po f", pi=128)
            hs_sb = cst.tile([128, KB, L], DT_PROJ, tag="hs")
            nc.sync.dma_start(out=hs_sb[:, 0:1, 0:512], in_=hs_re[:, 0:1, 0:512])
            nc.sync.dma_start(out=hs_sb[:, 0:1, 512:1024], in_=hs_re[:, 0:1, 512:1024])
            nc.sync.dma_start(out=hs_sb[:, 1:3, :], in_=hs_re[:, 1:3, :])
            nc.sync.dma_start(out=hs_sb[:, 3:6, :], in_=hs_re[:, 3:6, :])
            wqv_sb = cst.tile([128, KB, 256], DT_PROJ, tag="wqv")
            nc.scalar.dma_start(out=wqv_sb, in_=wqv_re)
            wk_sb = wqv_sb[:, :, 0:64]
            wq_sb = wqv_sb[:, :, 64:128]
            wv_sb = wqv_sb[:, :, 128:256]
            call_sb = cst.tile([128, 1344], DT_ATT, tag="call")
            nc.scalar.dma_start(out=call_sb, in_=c_all.ap())
            tri_sb = call_sb[:, 0:128]
            ident_sb = call_sb[:, 128:256]
            ones8_sb = call_sb[:, 256:320]
            sel_sb = call_sb[0:8, 320:1344]
            # wo as two base-0 tiles (o-proj lhsT per head)
            wo_sb = []
            for h in range(2):
                t = cst.tile([64, D], DT_OUT, tag=f"wo{h}", name=f"wo{h}")
                nc.scalar.dma_start(out=t, in_=wo.ap()[64 * h : 64 * h + 64, :])
                wo_sb.append(t)
            bias_sb = cst.tile([128, 1], F32, tag="bias")
            nc.vector.memset(bias_sb, A_BIAS)
            # row of ones at partition 64, for the den-reciprocal broadcast
            ones64_sb = cst.tile([65, 64], F32, tag="ones64")
            nc.vector.memset(ones64_sb, 0.0)
            nc.vector.memset(ones64_sb[64:65, :], 1.0)

            kq_sb = cst.tile([64, 2048], DT_ATT, tag="kq")
            vT_sb = cst.tile([128, L], DT_ATT, tag="vT")
            vx_sb = cst.tile([128, NCH, 130], DT_ATT, tag="vx")
            colsum_sb = cst.tile([8, 130], DT_ATT, tag="colsum")
            yT_sb = [
                cst.tile([64, L], DT_OUT, tag=f"yT{h}", name=f"yT{h}") for h in range(2)
            ]
            outT_sb = cst.tile([128, KB, L], DT_OUT, tag="outT")

            # ================= warmup + projections =================
            with tc.tile_pool(name="ps1", bufs=1, space="PSUM") as ps1:
                pqk = {}
                for half in range(2):
                    for coff in (0, 1024):
                        pqk[(half, coff)] = ps1.tile(
                            [64, 512],
                            F32,
                            tag=f"pqk{coff}_{half}",
                            name=f"pqk{coff}_{half}",
                        )
                pvT = [
                    ps1.tile([128, 512], F32, tag=f"pvT{i}", name=f"pvT{i}")
                    for i in range(2)
                ]

                # dummy matmuls into the qk psum banks: real K=128 array
                # activity that trips the HAM clock gate to 8/8 while the
                # input DMAs land. Overwritten by the first real qk MM.
                slots = list(pqk.values())
                for i in range(N_WARM):
                    nc.tensor.matmul(
                        slots[i % 4][:, 0:N_WARM_COLS],
                        warm_st,
                        warm_mv,
                        start=True,
                        stop=True,
                    )

                # q/k -> kq_sb [64, 2048]; partitions 0-15 head0, 32-47
                # head1 (rest zero); cols 0-1023 = k^T, 1024-2047 = q^T
                for kb in range(KB):
                    for half in range(2):
                        for w_sb, coff in ((wk_sb, 0), (wq_sb, 1024)):
                            nc.tensor.matmul(
                                pqk[(half, coff)],
                                w_sb[:, kb, :],
                                hs_sb[:, kb, half * 512 : (half + 1) * 512],
                                start=(kb == 0),
                                stop=(kb == KB - 1),
                            )
                    # v^T accumulation: wv-stationary, [c=128, t] layout
                    for half in range(2):
                        nc.tensor.matmul(
                            pvT[half],
                            wv_sb[:, kb, :],
                            hs_sb[:, kb, half * 512 : (half + 1) * 512],
                            start=(kb == 0),
                            stop=(kb == KB - 1),
                        )
                for half in range(2):
                    for coff in (0, 1024):
                        nc.vector.tensor_copy(
                            kq_sb[:, coff + half * 512 : coff + (half + 1) * 512],
                            pqk[(half, coff)],
                        )
                    nc.vector.tensor_copy(
                        vT_sb[:, half * 512 : (half + 1) * 512], pvT[half]
                    )

            # transpose v^T -> vx [kv, c] per 128-chunk (ping-ponged);
            # ones cols ride at 64 / 129 for the denominator
            with tc.tile_pool(name="ps2", bufs=2, space="PSUM") as ps2:
                nc.vector.memset(vx_sb[:, :, 64], 1.0)
                nc.vector.memset(vx_sb[:, :, 129], 1.0)
                for ch in range(NCH):
                    ptp = ps2.tile([128, 128], DT_ATT, tag="ptp", name=f"ptp{ch}")
                    nc.tensor.transpose(
                        ptp, vT_sb[:, ch * 128 : (ch + 1) * 128], ident_sb
                    )
                    nc.vector.tensor_copy(vx_sb[:, ch, 0:64], ptp[:, 0:64])
                    nc.vector.tensor_copy(vx_sb[:, ch, 65:129], ptp[:, 64:128])

                # per-chunk column sums of vx (inter-chunk +1/2 term)
                pcs = ps2.tile([8, 130], F32, tag="pcs", name="pcs")
                for ch in range(NCH):
                    nc.tensor.matmul(
                        pcs,
                        ones8_sb[:, ch * 8 : (ch + 1) * 8],
                        vx_sb[:, ch, :],
                        start=(ch == 0),
                        stop=(ch == NCH - 1),
                    )
                nc.vector.tensor_copy(colsum_sb, pcs)

            # ================= attention, half-L streamed =================
            with (
                tc.tile_pool(name="psnum", bufs=1, space="PSUM") as psnum,
                tc.tile_pool(name="psa", bufs=2, space="PSUM") as psa,
                tc.tile_pool(name="pso", bufs=2, space="PSUM") as pso,
            ):
                nums = [
                    psnum.tile([65, L], F32, tag=f"pN{h}", name=f"num{h}")
                    for h in range(2)
                ]

                def chunk_scores(j):
                    """Scores/num contributions of kv-chunk j to all t >=
                    j*128, including the intra-chunk +1/2 causal term (via
                    the diagonal-block DVE ops)."""
                    tlo = j * 128
                    for h in range(2):
                        sq = sqp.tile([128, 1024], DT_ATT, tag="sq", name=f"sq{j}_{h}")
                        for a, b in _bank_splits(tlo, L):
                            w = b - a
                            pa = psa.tile(
                                [128, 512], F32, tag="pA", name=f"pa{j}_{h}_{a}"
                            )[:, :w]
                            nc.tensor.matmul(
                                pa,
                                kq_sb[32 * h : 32 * h + 32, tlo : tlo + 128],
                                kq_sb[32 * h : 32 * h + 32, 1024 + a : 1024 + b],
                                start=True,
                                stop=True,
                            )
                            sqs = sq[:, a - tlo : b - tlo]
                            nc.scalar.activation(
                                out=sqs,
                                in_=pa,
                                func=mybir.ActivationFunctionType.Square,
                                scale=A_SCALE,
                                bias=bias_sb,
                            )
                            if a == tlo:
                                # diagonal block: causal mask + the +1/2
                                # intra-chunk term, (sq + 1/2) * tri
                                nc.vector.scalar_tensor_tensor(
                                    out=sqs[:, 0:128],
                                    in0=sqs[:, 0:128],
                                    scalar=0.5,
                                    in1=tri_sb,
                                    op0=mybir.AluOpType.add,
                                    op1=mybir.AluOpType.mult,
                                )
                            # num^T += V_j^T-stationary @ sq
                            nc.tensor.matmul(
                                nums[h][:, a:b],
                                vx_sb[:, j, 65 * h : 65 * h + 65],
                                sqs,
                                start=(j == 0),
                                stop=False,
                            )

                def half_finalize_num(half):
                    """Close the num accumulation for t-half `half`: add the
                    inter-chunk +1/2 term, then divide by den (row 64)."""
                    lo = half * 512
                    for h in range(2):
                        nc.tensor.matmul(
                            nums[h][:, lo : lo + 512],
                            colsum_sb[:, 65 * h : 65 * h + 65],
                            sel_sb[:, lo : lo + 512],
                            start=False,
                            stop=True,
                        )
                        rc = wrk.tile([65, 512], F32, tag="rc")
                        nc.vector.reciprocal_approx_fast(
                            out=rc, in_=nums[h][:, lo : lo + 512]
                        )
                        prb = pso.tile([128, 512], F32, tag="po", name=f"prb{half}_{h}")
                        nc.tensor.matmul(
                            prb[0:64, :],
                            ones64_sb[64:65, :],
                            rc[64:65, :],
                            start=True,
                            stop=True,
                        )
                        rb = wrk.tile([64, 512], F32, tag="rb")
                        nc.vector.tensor_copy(rb, prb[0:64, :])
                        nc.vector.tensor_mul(
                            yT_sb[h][:, lo : lo + 512],
                            nums[h][0:64, lo : lo + 512],
                            rb,
                        )

                def half_oproj(half):
                    """out^T[e, t-half] = sum_h Wo_h^T yT_h; stream to HBM."""
                    lo = half * 512
                    for e in range(KB):
                        po = pso.tile([128, 512], F32, tag="po", name=f"po{half}_{e}")
                        for h in range(2):
                            nc.tensor.matmul(
                                po,
                                wo_sb[h][:, e * 128 : (e + 1) * 128],
                                yT_sb[h][:, lo : lo + 512],
                                start=(h == 0),
                                stop=(h == 1),
                            )
                        if e % 2 == 0:
                            nc.vector.tensor_copy(outT_sb[:, e, lo : lo + 512], po)
                        else:
                            nc.scalar.copy(outT_sb[:, e, lo : lo + 512], po)
                    outT_re = outT.ap().rearrange("(po pi) t -> pi po t", pi=128)
                    nc.sync.dma_start(
                        out=outT_re[:, :, lo : lo + 512],
                        in_=outT_sb[:, :, lo : lo + 512],
                    )

                # software pipeline: finalize each t-half as soon as its last
                # kv-chunk lands, o-proj overlapped with later score chunks
                for j in range(NCH):
                    chunk_scores(j)
                    if j == 4:
                        half_finalize_num(0)
                    if j == 5:
                        half_oproj(0)
                half_finalize_num(1)
                half_oproj(1)

    nc.finalize()
    return nc


def _host_consts():
    s = np.arange(128)[:, None]
    t = np.arange(128)[None, :]
    tri = (s <= t).astype(np.float32)
    sel = np.zeros((8, 1024), dtype=np.float32)
    for i in range(8):
        sel[:i, i * 128 : (i + 1) * 128] = 0.5
    ident = np.eye(128, dtype=np.float32)
    ones8 = np.zeros((128, 64), dtype=np.float32)
    for ch in range(8):
        ones8[:, ch * 8 + ch] = 1.0
    return tri, sel, ident, ones8


def kernel(hidden_states, Wq, Wk, Wv, Wo):
    global _compiled_nc, _last_in_maps
    hs = np.asarray(hidden_states, dtype=np.float32)[0]  # [L, D]
    Wq = np.asarray(Wq, dtype=np.float32)
    Wk = np.asarray(Wk, dtype=np.float32)
    Wv = np.asarray(Wv, dtype=np.float32)
    Wo = np.asarray(Wo, dtype=np.float32)

    if _compiled_nc is None:
        _compiled_nc = _build_nc()
    nc = _compiled_nc

    proj_dt = _np_dt(DT_PROJ)
    att_dt = _np_dt(DT_ATT)
    out_dt = _np_dt(DT_OUT)

    hsT = np.ascontiguousarray(hs.T).astype(proj_dt)  # [D, L]
    tri, sel, ident, ones8 = _host_consts()
    c_all = np.zeros((128, 1344), dtype=np.float32)
    c_all[:, 0:128] = tri
    c_all[:, 128:256] = ident
    c_all[:, 256:320] = ones8
    c_all[0:8, 320:1344] = sel
    c_all = c_all.astype(att_dt)

    in_maps = []
    for c in range(NCORE):
        heads = [2 * c, 2 * c + 1]
        wk_c = np.zeros((D, 64), dtype=np.float32)
        wq_c = np.zeros((D, 64), dtype=np.float32)
        wv_c = np.zeros((D, 128), dtype=np.float32)
        wo_c = np.zeros((128, D), dtype=np.float32)
        for hi, h in enumerate(heads):
            if h >= H:
                continue
            wk_c[:, 32 * hi : 32 * hi + FD] = Wk[:, h * FD : (h + 1) * FD]
            wq_c[:, 32 * hi : 32 * hi + FD] = Wq[:, h * FD : (h + 1) * FD]
            wv_c[:, 64 * hi : 64 * hi + HD] = Wv[:, h * HD : (h + 1) * HD]
            wo_c[64 * hi : 64 * hi + HD, :] = Wo[h * HD : (h + 1) * HD, :]
        wqv_c = np.concatenate([wk_c, wq_c, wv_c], axis=1)
        in_maps.append(
            {
                "hsT": hsT,
                "wqv": wqv_c.astype(proj_dt),
                "wo": wo_c.astype(out_dt),
                "c_all": c_all,
            }
        )

    _last_in_maps = in_maps
    res = run_bass_kernel_spmd(nc, in_maps, list(range(NCORE)))
    acc = np.zeros((L, D), dtype=np.float32)
    for c in range(NCORE):
        acc += np.asarray(res.results[c]["outT"], dtype=np.float32).T
    return acc.reshape(1, L, D)
